# revision 1
# baseline (speedup 1.0000x reference)
"""Multi-head self-attention (RoPE, causal) on 8 Trainium2 NeuronCores.

Sharding: tensor-parallel over heads. Each core owns 2 of 16 heads:
  - QKV projections column-sharded (each core computes its 128 features)
  - attention per (batch, head) pair fully on-core, scores kept in the
    transposed orientation [tk, tq] so softmax needs no PE transposes:
    exp on ACT, denominator via a ones-row appended to V, causal handled
    block-wise + a triangular mask on diagonal blocks
  - AllToAll switches from head-sharding to token-sharding (4MB/core)
  - output projection token-sharded, output written in natural layout

dtypes: fp32r (TF32-like, full PE rate at N>=512) everywhere except the
softmax probabilities / V operand of the PV matmul, which are bf16.
"""

import numpy as np
import ml_dtypes

import concourse.bacc as bacc
import concourse.mybir as mybir
import concourse.tile as tile
from concourse import bass_utils
from concourse.masks import make_identity

F32 = mybir.dt.float32
F32R = mybir.dt.float32r
BF16 = mybir.dt.bfloat16

B, T, D = 4, 2048, 1024
H, DH = 16, 64
N_CORES = 8
HPC = H // N_CORES            # heads per core = 2
EC = HPC * DH                 # feature slice per core = 128
NT = B * T                    # 8192 tokens
TPC = NT // N_CORES           # 1024 tokens per core
THETA = 10000.0

_CACHE = {}
last_results = None  # BassKernelResults of the most recent run (for profiling)


def _build_program():
    nc = bacc.Bacc("TRN2", debug=False, target_bir_lowering=False,
                   num_devices=N_CORES)

    xt_d = nc.dram_tensor("xt", [128, 8, NT], BF16, kind="ExternalInput")
    wq_d = nc.dram_tensor("wq", [128, 8, EC], BF16, kind="ExternalInput")
    wk_d = nc.dram_tensor("wk", [128, 8, EC], BF16, kind="ExternalInput")
    wv_d = nc.dram_tensor("wv", [128, 8, EC], BF16, kind="ExternalInput")
    wo_d = nc.dram_tensor("wo", [128, 8, D], BF16, kind="ExternalInput")
    cos_d = nc.dram_tensor("cosb", [128, T], F32, kind="ExternalInput")
    sin_d = nc.dram_tensor("sinb", [128, T], F32, kind="ExternalInput")
    rotm_d = nc.dram_tensor("rotm", [128, 128], F32R, kind="ExternalInput")
    tri_d = nc.dram_tensor("trimask", [128, 128], BF16, kind="ExternalInput")
    y_d = nc.dram_tensor("y", [TPC, D], F32, kind="ExternalOutput")

    NB = T // 128      # 16 tk blocks per batch
    NCHUNK = NT // 512  # 16 phase-1 chunks

    with tile.TileContext(nc) as tc:
        with (
            tc.tile_pool(name="consts", bufs=1) as consts,
            tc.tile_pool(name="wpool", bufs=1) as wpool,
            tc.tile_pool(name="big", bufs=1) as big,
            tc.tile_pool(name="xp", bufs=2) as xp,
            tc.tile_pool(name="stage", bufs=2) as stage,
            tc.tile_pool(name="expp", bufs=4) as expp,
            tc.tile_pool(name="outp", bufs=2) as outp,
            tc.tile_pool(name="psA", bufs=1, space="PSUM") as psA,
            tc.tile_pool(name="psB", bufs=1, space="PSUM") as psB,
            tc.tile_pool(name="pvA", bufs=2, space="PSUM") as pvA,
            tc.tile_pool(name="pvB", bufs=2, space="PSUM") as pvB,
            tc.tile_pool(name="dram", bufs=2, space="DRAM") as dram,
        ):
            # ---- constants ----
            cos_sb = consts.tile([128, T], F32)
            sin_sb = consts.tile([128, T], F32)
            rotm_sb = consts.tile([128, 128], F32R)
            tri_sb = consts.tile([128, 128], BF16)
            ident_sb = consts.tile([128, 128], F32)
            nc.sync.dma_start(cos_sb[:], cos_d[:, :])
            nc.sync.dma_start(sin_sb[:], sin_d[:, :])
            nc.sync.dma_start(rotm_sb[:], rotm_d[:, :])
            nc.sync.dma_start(tri_sb[:], tri_d[:, :])
            make_identity(nc, ident_sb[:])

            wq_sb = consts.tile([128, 8, EC], BF16)
            wk_sb = consts.tile([128, 8, EC], BF16)
            wv_sb = consts.tile([128, 8, EC], BF16)
            nc.sync.dma_start(wq_sb[:], wq_d[:, :, :])
            nc.sync.dma_start(wk_sb[:], wk_d[:, :, :])
            nc.sync.dma_start(wv_sb[:], wv_d[:, :, :])

            # ---- persistent tensors ----
            qT = big.tile([128, NT], F32R, tag="qT")
            kT = big.tile([128, NT], F32R, tag="kT")
            # V per (pair, tk-block): [tk=128, 65] with ones in col 64
            vext = big.tile([128, HPC * B, NB, 65], BF16, tag="vext")
            nc.vector.memset(vext[:, :, :, 64], 1.0)

            a2a_in1 = dram.tile([N_CORES, 128, 768], BF16)
            a2a_out1 = dram.tile([N_CORES, 128, 768], BF16)
            a2a_in2 = dram.tile([N_CORES, 128, 256], BF16)
            a2a_out2 = dram.tile([N_CORES, 128, 256], BF16)

            # ================= Phase 1: QKV projections + RoPE =============
            def do_chunk(ci):
                t0 = 512 * ci
                bb = t0 // T
                s0 = t0 % T
                xt = xp.tile([128, 8, 512], BF16, tag="x")
                nc.sync.dma_start(xt[:], xt_d[:, :, t0:t0 + 512])

                # pipelined: proj(q) -> ACTcopy(q) -> proj(k) -> rot(q) ->
                # ACTcopy(k) -> proj(v) -> rot(k) -> ACTcopy(v) -> vtrans
                # so PE never sits behind an ACT drain.
                def _proj(w_sb, nm, pool):
                    pt = pool.tile([128, 1024], F32, tag="s", name="p" + nm)
                    pt = pt[:, 0:512]
                    for ko in range(8):
                        nc.tensor.matmul(pt, w_sb[:, ko, :], xt[:, ko, :],
                                         start=(ko == 0), stop=(ko == 7))
                    return pt

                def _rot(raw, nm, pool):
                    rot = pool.tile([128, 1024], F32, tag="s", name="r" + nm)
                    rot = rot[:, 0:512]
                    nc.tensor.matmul(rot, rotm_sb[:], raw[:],
                                     start=True, stop=True)
                    return rot

                def _rope_combine(raw, rot, dest):
                    t1 = stage.tile([128, 512], F32, tag="t1")
                    nc.vector.tensor_tensor(
                        t1[:], raw[:], cos_sb[:, s0:s0 + 512],
                        mybir.AluOpType.mult)
                    t2 = stage.tile([128, 512], F32, tag="t2")
                    nc.vector.tensor_tensor(
                        t2[:], rot[:], sin_sb[:, s0:s0 + 512],
                        mybir.AluOpType.mult)
                    nc.vector.tensor_tensor(
                        dest[:, t0:t0 + 512], t1[:], t2[:],
                        mybir.AluOpType.add)

                pq = _proj(wq_sb, "q", psA)
                rawq = stage.tile([128, 512], F32R, tag="rawq")
                nc.scalar.copy(rawq[:], pq)
                pk = _proj(wk_sb, "k", psB)
                rotq = _rot(rawq, "q", psA)
                rawk = stage.tile([128, 512], F32R, tag="rawk")
                nc.scalar.copy(rawk[:], pk)
                pv_ = _proj(wv_sb, "v", psB)
                rotk = _rot(rawk, "k", psA)
                vraw = stage.tile([128, 512], F32, tag="vraw")
                nc.scalar.copy(vraw[:], pv_)
                _rope_combine(rawq, rotq, qT)
                _rope_combine(rawk, rotk, kT)
                for h in range(HPC):
                    pair = bb * HPC + h
                    for bi in range(4):
                        jg = s0 // 128 + bi
                        tp = (psB if bi % 2 else psA).tile(
                            [128, 1024], F32, tag="s", name="vtr")[:, 0:64]
                        nc.tensor.transpose(
                            tp,
                            vraw[64 * h:64 * h + 64,
                                 128 * bi:128 * bi + 128],
                            ident_sb[64 * h:64 * h + 64,
                                     64 * h:64 * h + 64])
                        nc.vector.tensor_copy(
                            vext[:, pair, jg, 0:64], tp)

            # ================= Phase 2: attention ==========================
            # Two heads of the same batch run as interleaved pipeline
            # streams: ACT-exp latency of one stream hides behind PE work
            # of the other.
            def do_attn(bb):
                tb0 = bb * T
                qs = [qT[64 * hh:64 * hh + 64, tb0:tb0 + T] for hh in range(2)]
                ks = [kT[64 * hh:64 * hh + 64, tb0:tb0 + T] for hh in range(2)]
                spools = [psA, psB]
                vpools = [pvA, pvB]
                for c2 in range(2):
                    jmax = 8 * (c2 + 1)
                    pvt = [[vpools[hh].tile(
                        [65, 512], F32, tag="pv",
                        name=f"pv_{bb}_{hh}_{c2}_{hf}") for hf in range(2)]
                        for hh in range(2)]

                    def _scores_pair(j):
                        # both heads' score matmuls, issued alternating so
                        # the two K=64 row-strips (partitions 0-63 / 64-127)
                        # execute concurrently in the PE array.
                        spts = [spools[hh].tile(
                            [128, 1024], F32, tag="s",
                            name=f"s_{bb}_{hh}_{c2}_{j}") for hh in range(2)]
                        for hf in range(2):
                            cl0 = 1024 * c2 + 512 * hf
                            if cl0 + 512 <= 128 * j:
                                continue
                            w = cl0 + 512 - max(cl0, 128 * j)
                            N = 512 if w == 512 else max(256, w)
                            st = cl0 + 512 - N
                            for hh in range(2):
                                nc.tensor.matmul(
                                    spts[hh][:, st - 1024 * c2:
                                             st - 1024 * c2 + N],
                                    ks[hh][:, 128 * j:128 * j + 128],
                                    qs[hh][:, st:st + N],
                                    start=True, stop=True)
                        return spts

                    def _pv(j, exs):
                        lo = max(0, 128 * j - 1024 * c2)
                        for hh in range(2):
                            vt = vext[:, bb * HPC + hh, j, :]
                            for hf in range(2):
                                h0 = 512 * hf
                                a = max(h0, lo)
                                if a < h0 + 512:
                                    last_j = min(jmax - 1,
                                                 8 * c2 + 4 * hf + 3)
                                    nc.tensor.matmul(
                                        pvt[hh][hf][:, a - h0:512],
                                        vt, exs[hh][:, a:h0 + 512],
                                        start=(j == 0), stop=(j == last_j))

                    # software pipeline with one-iteration PV delay: the PE
                    # always has ready work (PV of j-1) at its queue head
                    # while ACT computes exp(j).
                    spt = _scores_pair(0)
                    prev = None
                    for j in range(jmax):
                        lo = max(0, 128 * j - 1024 * c2)
                        exs = []
                        for hh in range(2):
                            ex = expp.tile([128, 1024], BF16, tag="e",
                                           name=f"e_{hh}")
                            nc.scalar.activation(
                                ex[:, lo:1024], spt[hh][:, lo:1024],
                                mybir.ActivationFunctionType.Exp, scale=0.125)
                            exs.append(ex)
                        if prev is not None:
                            _pv(prev[0], prev[1])
                        if j + 1 < jmax:
                            spt = _scores_pair(j + 1)
                        for hh in range(2):
                            if 128 * j >= 1024 * c2:
                                nc.vector.tensor_tensor(
                                    exs[hh][:, lo:lo + 128],
                                    exs[hh][:, lo:lo + 128],
                                    tri_sb[:], mybir.AluOpType.mult)
                        prev = (j, exs)
                    _pv(prev[0], prev[1])
                    # normalize + ship to a2a_in.  Copy psum out first
                    # (ACT) so the pv slots free up for the next chunk.
                    for hh in range(2):
                        dnm = outp.tile([33, 512], F32, tag="dnm")
                        unn = [None, None]
                        for hf in range(2):
                            nc.vector.tensor_copy(
                                dnm[32 * hf:32 * hf + 1, :],
                                pvt[hh][hf][64:65, :])
                            unn[hf] = outp.tile([64, 512], BF16,
                                                tag=f"unn{hf}",
                                                name=f"unn{hf}")
                            nc.scalar.copy(unn[hf][:], pvt[hh][hf][0:64, :])
                        rec = outp.tile([33, 512], F32, tag="rec")
                        nc.vector.reciprocal(rec[:], dnm[:])
                        rscr = dram.tile([2, 512], F32, tag="rscr",
                                         name="rscr")
                        for hf in range(2):
                            nc.sync.dma_start(rscr[hf:hf + 1, :],
                                              rec[32 * hf:32 * hf + 1, :])
                        for hf in range(2):
                            recb = outp.tile([64, 512], F32, tag="recb")
                            nc.sync.dma_start(
                                recb[:],
                                rscr[hf:hf + 1, :].to_broadcast((64, 512)))
                            ao = outp.tile([64, 512], BF16, tag="ao")
                            nc.vector.tensor_tensor(
                                ao[:], unn[hf][:], recb[:],
                                mybir.AluOpType.mult)
                            # group 1 = batches 0-2 (768 tokens/dest),
                            # group 2 = batch 3 (256 tokens/dest)
                            if bb < 3:
                                grp, base, W = a2a_in1, 0, 768
                            else:
                                grp, base, W = a2a_in2, 6144, 256
                            tt = 2048 * bb + 1024 * c2 + 512 * hf - base
                            off = 0
                            while off < 512:
                                dd = (tt + off) // W
                                col = (tt + off) % W
                                w = min(512 - off, W - col)
                                nc.sync.dma_start(
                                    grp[dd, 64 * hh:64 * hh + 64,
                                        col:col + w],
                                    ao[:, off:off + w])
                                off += w

            def do_oproj(g, oall_g, row0, ntb):
                # y rows [row0, row0 + 128*ntb) from group-g tokens
                for eo in range(2):
                    wo_sb = wpool.tile([128, 8, 512], BF16, tag="wo",
                                       name=f"wo_{g}_{eo}")
                    nc.sync.dma_start(wo_sb[:],
                                      wo_d[:, :, 512 * eo:512 * eo + 512])
                    for tb in range(ntb):
                        ot = (psB if (tb + eo) % 2 else psA).tile(
                            [128, 1024], F32, tag="s", name="ot")[:, 0:512]
                        for ec in range(8):
                            nc.tensor.matmul(
                                ot, oall_g[:, ec, 128 * tb:128 * tb + 128],
                                wo_sb[:, ec, :],
                                start=(ec == 0), stop=(ec == 7))
                        ys = outp.tile([128, 512], F32, tag="y")
                        nc.scalar.copy(ys[:], ot)
                        nc.sync.dma_start(
                            y_d[row0 + 128 * tb:row0 + 128 * tb + 128,
                                512 * eo:512 * eo + 512], ys[:])

            # interleave phase 1 and attention per batch; group-0 A2A and
            # its output projection overlap batches 2-3.
            rg = [list(range(N_CORES))]
            for bb in range(3):
                for ci in range(4 * bb, 4 * bb + 4):
                    do_chunk(ci)
                do_attn(bb)
            nc.gpsimd.collective_compute(
                "AllToAll", mybir.AluOpType.bypass, replica_groups=rg,
                ins=[a2a_in1.opt()], outs=[a2a_out1.opt()])
            oall1 = wpool.tile([128, 8, 768], BF16, tag="oall1")
            nc.sync.dma_start(oall1[:],
                              a2a_out1[:].rearrange("s p t -> p s t"))
            for ci in range(12, 16):
                do_chunk(ci)
            do_attn(3)
            nc.gpsimd.collective_compute(
                "AllToAll", mybir.AluOpType.bypass, replica_groups=rg,
                ins=[a2a_in2.opt()], outs=[a2a_out2.opt()])
            do_oproj(0, oall1, 0, 6)
            oall2 = wpool.tile([128, 8, 256], BF16, tag="oall2")
            nc.sync.dma_start(oall2[:],
                              a2a_out2[:].rearrange("s p t -> p s t"))
            do_oproj(1, oall2, 768, 2)

    nc.compile()
    return nc


def _host_inputs(x, Wq, Wk, Wv, Wo, token_positions):
    """Per-core in_maps with transposed/tiled layouts."""
    x = np.asarray(x, dtype=np.float32)
    xt_bf = np.ascontiguousarray(
        x.reshape(NT, D).T.reshape(8, 128, NT).transpose(1, 0, 2)
    ).astype(ml_dtypes.bfloat16)

    pos = np.asarray(token_positions).astype(np.float64)
    inv_freq = 1.0 / (THETA ** (np.arange(0, DH, 2, dtype=np.float64) / DH))
    ang = pos[None, :] * inv_freq[:, None]          # [32, T]
    cos_p = np.cos(ang)                              # pair i
    sin_p = np.sin(ang)
    # partition p (0..127): within-head dim d = p % 64, pair = d // 2
    d_idx = (np.arange(128) % 64) // 2
    cosb = cos_p[d_idx, :].astype(np.float32)
    sinb = sin_p[d_idx, :].astype(np.float32)

    rotm = np.zeros((128, 128), dtype=np.float32)
    for i in range(64):
        rotm[2 * i + 1, 2 * i] = -1.0   # out[2i] -= in[2i+1]*sin -> rot[2i] = -in[2i+1]
        rotm[2 * i, 2 * i + 1] = 1.0    # rot[2i+1] = in[2i]
    tri = np.tril(np.ones((128, 128), dtype=np.float32)).T  # [tk, tq] tk<=tq
    tri = tri.astype(ml_dtypes.bfloat16)

    def wtiles(W, sl):
        # lhsT tiles: [p, ko, e] with d = ko*128+p contracting
        Wt = np.ascontiguousarray(W[sl, :].T)        # [D, e]
        return np.ascontiguousarray(
            Wt.reshape(8, 128, Wt.shape[1]).transpose(1, 0, 2))

    WoT = np.ascontiguousarray(np.asarray(Wo, dtype=np.float32).T)  # [e_in, e_out]
    wo_t = np.ascontiguousarray(WoT.reshape(8, 128, D).transpose(1, 0, 2))

    in_maps = []
    for c in range(N_CORES):
        sl = slice(EC * c, EC * (c + 1))
        in_maps.append({
            "xt": xt_bf,
            "wq": wtiles(np.asarray(Wq, np.float32), sl).astype(ml_dtypes.bfloat16),
            "wk": wtiles(np.asarray(Wk, np.float32), sl).astype(ml_dtypes.bfloat16),
            "wv": wtiles(np.asarray(Wv, np.float32), sl).astype(ml_dtypes.bfloat16),
            "wo": wo_t.astype(ml_dtypes.bfloat16),
            "cosb": cosb,
            "sinb": sinb,
            "rotm": rotm,
            "trimask": tri,
        })
    return in_maps


def kernel(x, Wq, Wk, Wv, Wo, token_positions):
    global last_results
    if "nc" not in _CACHE:
        _CACHE["nc"] = _build_program()
    nc = _CACHE["nc"]
    in_maps = _host_inputs(x, Wq, Wk, Wv, Wo, token_positions)
    res = bass_utils.run_bass_kernel_spmd(nc, in_maps, list(range(N_CORES)))
    last_results = res
    y = np.empty((NT, D), dtype=np.float32)
    for c in range(N_CORES):
        yc = res.results[c]["y"]
        y[768 * c:768 * c + 768] = yc[0:768]
        y[6144 + 256 * c:6144 + 256 * c + 256] = yc[768:1024]
    return y.reshape(B, T, D)



# revision 3
# speedup vs baseline: 1.4834x; 1.4834x over previous
"""Multi-head self-attention (RoPE, causal) on 8 Trainium2 NeuronCores.

Sharding: tensor-parallel over heads. Each core owns 2 of 16 heads:
  - QKV projections column-sharded (each core computes its 128 features)
  - attention per (batch, head) fully on-core; scores kept transposed
    [tk, tq] so softmax needs no PE transposes; exp on ACT (both heads in
    one instruction via a strided AP); denominator via a ones-row in V;
    normalization via approx-reciprocal + gpsimd partition-broadcast
  - per-batch AllToAll (strided token blocks) switches head-sharding to
    token-sharding; output projection runs per batch, overlapped with the
    next batch's attention
  - phase-1 chunks for batch b+1 and oproj for batch b-1 are interleaved
    into batch b's attention loop so the PE never starves

dtypes: bf16 for all matmul operands (q/k/v, probs, weights); fp32 PSUM
and softmax statistics.
"""

import numpy as np
import ml_dtypes

import concourse.bacc as bacc
import concourse.mybir as mybir
import concourse.tile as tile
from concourse import bass_utils

F32 = mybir.dt.float32
BF16 = mybir.dt.bfloat16

B, T, D = 4, 2048, 1024
H, DH = 16, 64
N_CORES = 8
HPC = H // N_CORES            # heads per core = 2
EC = HPC * DH                 # feature slice per core = 128
NT = B * T                    # 8192 tokens
TPC = NT // N_CORES           # 1024 tokens per core
THETA = 10000.0
NBB = T // 128                # 16 tk blocks per batch

_CACHE = {}
last_results = None


def _build_program():
    nc = bacc.Bacc("TRN2", debug=False, target_bir_lowering=False,
                   num_devices=N_CORES)

    xt_d = nc.dram_tensor("xt", [128, 8, NT], BF16, kind="ExternalInput")
    wq_d = nc.dram_tensor("wq", [128, 8, EC], BF16, kind="ExternalInput")
    wk_d = nc.dram_tensor("wk", [128, 8, EC], BF16, kind="ExternalInput")
    wv_d = nc.dram_tensor("wv", [128, 8, EC], BF16, kind="ExternalInput")
    wo_d = nc.dram_tensor("wo", [128, 8, D], BF16, kind="ExternalInput")
    cos_d = nc.dram_tensor("cosb", [128, T], F32, kind="ExternalInput")
    sin_d = nc.dram_tensor("sinb", [128, T], F32, kind="ExternalInput")
    rotm_d = nc.dram_tensor("rotm", [128, 128], BF16, kind="ExternalInput")
    tri_d = nc.dram_tensor("trimask", [128, 128], BF16, kind="ExternalInput")
    id_d = nc.dram_tensor("identb", [128, 128], BF16, kind="ExternalInput")
    y_d = nc.dram_tensor("y", [TPC, D], F32, kind="ExternalOutput")

    with tile.TileContext(nc) as tc:
        with (
            tc.tile_pool(name="consts", bufs=1) as consts,
            tc.tile_pool(name="wpool", bufs=1) as wpool,
            tc.tile_pool(name="big", bufs=1) as big,
            tc.tile_pool(name="xp", bufs=2) as xp,
            tc.tile_pool(name="stage", bufs=2) as stage,
            tc.tile_pool(name="expp", bufs=4) as expp,
            tc.tile_pool(name="outp", bufs=2) as outp,
            tc.tile_pool(name="oall_p", bufs=2) as oall_p,
            tc.tile_pool(name="scp", bufs=2, space="PSUM") as scp,
            tc.tile_pool(name="pvp", bufs=1, space="PSUM") as pvp,
            tc.tile_pool(name="psC", bufs=2, space="PSUM") as psC,
            tc.tile_pool(name="dram", bufs=1, space="DRAM") as dram,
        ):
            # ---- constants ----
            cos_sb = consts.tile([128, T], F32)
            sin_sb = consts.tile([128, T], F32)
            rotm_sb = consts.tile([128, 128], BF16)
            tri_sb = consts.tile([128, 128], BF16)
            ident_sb = consts.tile([128, 128], BF16)
            nc.sync.dma_start(cos_sb[:], cos_d[:, :])
            nc.sync.dma_start(sin_sb[:], sin_d[:, :])
            nc.sync.dma_start(rotm_sb[:], rotm_d[:, :])
            nc.sync.dma_start(tri_sb[:], tri_d[:, :])
            nc.sync.dma_start(ident_sb[:], id_d[:, :])

            wq_sb = consts.tile([128, 8, EC], BF16)
            wk_sb = consts.tile([128, 8, EC], BF16)
            wv_sb = consts.tile([128, 8, EC], BF16)
            wo_sb = consts.tile([128, 8, D], BF16)
            nc.sync.dma_start(wq_sb[:], wq_d[:, :, :])
            nc.sync.dma_start(wk_sb[:], wk_d[:, :, :])
            nc.sync.dma_start(wv_sb[:], wv_d[:, :, :])
            nc.sync.dma_start(wo_sb[:], wo_d[:, :, :])

            # ---- persistent tensors ----
            qT = big.tile([128, NT], BF16, tag="qT")
            kT = big.tile([128, NT], BF16, tag="kT")
            vext = big.tile([128, HPC * B, NBB, 65], BF16, tag="vext")
            nc.vector.memset(vext[:, :, :, 64], 1.0)

            a2a_in = [dram.tile([N_CORES, 128, 256], BF16, tag=f"ai{b}",
                                name=f"ai{b}") for b in range(B)]
            a2a_out = [dram.tile([N_CORES, 128, 256], BF16, tag=f"ao{b}",
                                 name=f"ao{b}") for b in range(B)]

            xts = {}

            def xt_load(ci):
                xts[ci] = xp.tile([128, 8, 512], BF16, tag="x",
                                  name=f"xt{ci}")
                nc.sync.dma_start(xts[ci][:], xt_d[:, :, 512 * ci:
                                                   512 * ci + 512])

            # ---------- phase-1 chunk as a list of closures -------------
            def chunk_pieces(ci):
                t0 = 512 * ci
                bb = t0 // T
                s0 = t0 % T
                ps = []
                st = {}

                def proj_mm(w_sb, nm, ko):
                    def f():
                        if ko == 0:
                            st[nm] = psC.tile([128, 512], F32, tag="pp",
                                              name="p" + nm)
                        nc.tensor.matmul(st[nm], w_sb[:, ko, :],
                                         xts[ci][:, ko, :],
                                         start=(ko == 0), stop=(ko == 7))
                    return f

                def drain(nm):
                    def f():
                        st["raw" + nm] = stage.tile([128, 512], BF16,
                                                    tag="raw" + nm,
                                                    name="raw" + nm)
                        nc.scalar.copy(st["raw" + nm][:], st[nm])
                    return f

                def rot(nm):
                    def f():
                        st["rot" + nm] = psC.tile([128, 512], F32, tag="pp",
                                                  name="rot" + nm)
                        nc.tensor.matmul(st["rot" + nm], rotm_sb[:],
                                         st["raw" + nm][:],
                                         start=True, stop=True)
                    return f

                def comb(nm, dest):
                    def f1():
                        st["t2" + nm] = stage.tile([128, 512], F32,
                                                   tag="t2" + nm,
                                                   name="t2" + nm)
                        nc.vector.tensor_tensor(
                            st["t2" + nm][:], st["rot" + nm],
                            sin_sb[:, s0:s0 + 512], mybir.AluOpType.mult)

                    def f2():
                        st["t1" + nm] = stage.tile([128, 512], F32,
                                                   tag="t1" + nm,
                                                   name="t1" + nm)
                        nc.vector.tensor_tensor(
                            st["t1" + nm][:], st["raw" + nm][:],
                            cos_sb[:, s0:s0 + 512], mybir.AluOpType.mult)

                    def f3():
                        nc.vector.tensor_tensor(
                            dest[:, t0:t0 + 512], st["t1" + nm][:],
                            st["t2" + nm][:], mybir.AluOpType.add)
                    return [f1, f2, f3]

                def vtrans(h, bi):
                    def f():
                        pair = bb * HPC + h
                        jg = s0 // 128 + bi
                        tp = psC.tile([128, 512], F32, tag="pp",
                                      name="vtr").bitcast(BF16)[:, 0:64]
                        nc.tensor.transpose(
                            tp, st["rawv"][64 * h:64 * h + 64,
                                           128 * bi:128 * bi + 128],
                            ident_sb[64 * h:64 * h + 64,
                                     64 * h:64 * h + 64])
                        nc.vector.tensor_copy(vext[:, pair, jg, 0:64], tp)
                    return f

                for ko in range(8):
                    ps.append(proj_mm(wq_sb, "q", ko))
                ps.append(drain("q"))
                for ko in range(8):
                    ps.append(proj_mm(wk_sb, "k", ko))
                ps.append(rot("q"))
                ps.extend(comb("q", qT))
                ps.append(drain("k"))
                for ko in range(8):
                    ps.append(proj_mm(wv_sb, "v", ko))
                ps.append(rot("k"))
                ps.extend(comb("k", kT))
                ps.append(drain("v"))
                for h in range(HPC):
                    for bi in range(4):
                        ps.append(vtrans(h, bi))
                return ps

            # ---------- output projection for one batch -----------------
            def oproj_pieces(bb):
                ps = []
                st = {}

                def load():
                    st["oall"] = oall_p.tile([128, 8, 256], BF16, tag="oall",
                                             name=f"oall{bb}")
                    nc.sync.dma_start(
                        st["oall"][:],
                        a2a_out[bb][:].rearrange("s p t -> p s t"))

                def piece(tb, eo):
                    def f():
                        ot = psC.tile([128, 512], F32, tag="pp", name="ot")
                        for ec in range(8):
                            nc.tensor.matmul(
                                ot, st["oall"][:, ec, 128 * tb:128 * tb + 128],
                                wo_sb[:, ec, 512 * eo:512 * eo + 512],
                                start=(ec == 0), stop=(ec == 7))
                        ys = outp.tile([128, 512], F32, tag="ys", name="ys")
                        nc.scalar.copy(ys[:], ot)
                        nc.sync.dma_start(
                            y_d[256 * bb + 128 * tb:256 * bb + 128 * tb + 128,
                                512 * eo:512 * eo + 512], ys[:])
                    return f

                ps.append(load)
                for tb in range(2):
                    for eo in range(2):
                        ps.append(piece(tb, eo))
                return ps

            # ---------- attention for one batch, with filler -------------
            def do_attn(bb, filler):
                fidx = [0]

                def pop_filler(k):
                    while k > 0 and fidx[0] < len(filler):
                        filler[fidx[0]]()
                        fidx[0] += 1
                        k -= 1

                tb0 = bb * T
                pair0 = bb * HPC

                for q4 in range(4):
                    jmax = 4 * q4 + 4
                    tq0 = tb0 + 512 * q4     # global col base of quarter
                    pvt = pvp.tile([65, 2, 512], F32, tag="pv", name="pvt")

                    def scores(j):
                        sc = scp.tile([128, 2, 512], F32, tag="sc",
                                      name="sc")
                        lo = max(0, 128 * j - 512 * q4)
                        for hh in range(2):
                            nc.tensor.matmul(
                                sc[:, hh, lo:512],
                                kT[64 * hh:64 * hh + 64,
                                   tb0 + 128 * j:tb0 + 128 * j + 128],
                                qT[64 * hh:64 * hh + 64,
                                   tq0 + lo:tq0 + 512],
                                start=True, stop=True)
                        return sc, lo

                    s_cur = scores(0)
                    prev = None
                    for j in range(jmax):
                        sc, lo = s_cur
                        ex = expp.tile([128, 2, 512], BF16, tag="ex",
                                       name="ex")
                        nc.scalar.activation(
                            ex[:, :, lo:512], sc[:, :, lo:512],
                            mybir.ActivationFunctionType.Exp, scale=0.125)
                        if prev is not None:
                            pj, plo, pex = prev
                            for hh in range(2):
                                nc.tensor.matmul(
                                    pvt[:, hh, plo:512],
                                    vext[:, pair0 + hh, pj, 0:65],
                                    pex[:, hh, plo:512],
                                    start=(pj == 0), stop=False)
                        if j + 1 < jmax:
                            s_cur = scores(j + 1)
                        if j >= 4 * q4:
                            d0 = 128 * (j - 4 * q4)
                            for hh in range(2):
                                nc.vector.tensor_tensor(
                                    ex[:, hh, d0:d0 + 128],
                                    ex[:, hh, d0:d0 + 128],
                                    tri_sb[:], mybir.AluOpType.mult)
                        prev = (j, lo, ex)
                        pop_filler(5)
                    pj, plo, pex = prev
                    for hh in range(2):
                        nc.tensor.matmul(
                            pvt[:, hh, plo:512],
                            vext[:, pair0 + hh, pj, 0:65],
                            pex[:, hh, plo:512],
                            start=(pj == 0), stop=True)

                    # drain + normalize + ship.  1/denom via ACT ln/exp in
                    # place on the denominator row; partition_broadcast needs
                    # a base-partition-0 source, so hop through a tiny DMA.
                    unn = outp.tile([65, 2, 512], F32, tag="unn", name="unn")
                    nc.scalar.copy(unn[:], pvt[:])
                    nc.scalar.activation(unn[64:65, :, :], unn[64:65, :, :],
                                         mybir.ActivationFunctionType.Ln)
                    nc.scalar.activation(unn[64:65, :, :], unn[64:65, :, :],
                                         mybir.ActivationFunctionType.Exp,
                                         scale=-1.0)
                    rec = outp.tile([1, 2, 512], F32, tag="rec", name="rec")
                    nc.sync.dma_start(rec[:], unn[64:65, :, :])
                    recb = outp.tile([64, 2, 512], F32, tag="recb",
                                     name="recb")
                    nc.gpsimd.partition_broadcast(recb[:], rec[:])
                    ao = outp.tile([64, 2, 512], BF16, tag="aot", name="aot")
                    nc.vector.scalar_tensor_tensor(
                        ao[:], unn[0:64, :, :], 1.0, recb[:],
                        mybir.AluOpType.mult, mybir.AluOpType.mult)
                    for hh in range(2):
                        for tb in range(4):
                            j16 = 4 * q4 + tb
                            dest = j16 % 8
                            slot = j16 // 8
                            nc.sync.dma_start(
                                a2a_in[bb][dest, 64 * hh:64 * hh + 64,
                                           128 * slot:128 * slot + 128],
                                ao[:, hh, 128 * tb:128 * tb + 128])
                    pop_filler(6)

                nc.gpsimd.collective_compute(
                    "AllToAll", mybir.AluOpType.bypass,
                    replica_groups=[list(range(N_CORES))],
                    ins=[a2a_in[bb].opt()], outs=[a2a_out[bb].opt()])
                pop_filler(len(filler))

            # ================= main schedule ==========================
            # prologue: chunks of batch 0
            xt_load(0)
            for ci in range(4):
                if ci + 1 < 16:
                    xt_load(ci + 1)
                for p in chunk_pieces(ci):
                    p()
            for bb in range(B):
                filler = []
                if bb + 1 < B:
                    for ci in range(4 * bb + 4, 4 * bb + 8):
                        filler.extend(chunk_pieces(ci))
                        if ci + 1 < 16:
                            filler.append(lambda c=ci + 1: xt_load(c))
                if bb - 1 >= 0:
                    filler.extend(oproj_pieces(bb - 1))
                do_attn(bb, filler)
            for p in oproj_pieces(B - 1):
                p()

    nc.compile()
    return nc


def _host_inputs(x, Wq, Wk, Wv, Wo, token_positions):
    """Per-core in_maps with transposed/tiled layouts."""
    x = np.asarray(x, dtype=np.float32)
    xt_bf = np.ascontiguousarray(
        x.reshape(NT, D).T.reshape(8, 128, NT).transpose(1, 0, 2)
    ).astype(ml_dtypes.bfloat16)

    pos = np.asarray(token_positions).astype(np.float64)
    inv_freq = 1.0 / (THETA ** (np.arange(0, DH, 2, dtype=np.float64) / DH))
    ang = pos[None, :] * inv_freq[:, None]          # [32, T]
    cos_p = np.cos(ang)
    sin_p = np.sin(ang)
    d_idx = (np.arange(128) % 64) // 2
    cosb = cos_p[d_idx, :].astype(np.float32)
    sinb = sin_p[d_idx, :].astype(np.float32)

    rotm = np.zeros((128, 128), dtype=np.float32)
    for i in range(64):
        rotm[2 * i + 1, 2 * i] = -1.0
        rotm[2 * i, 2 * i + 1] = 1.0
    rotm = rotm.astype(ml_dtypes.bfloat16)
    tri = np.tril(np.ones((128, 128), dtype=np.float32)).T  # [tk, tq]
    tri = tri.astype(ml_dtypes.bfloat16)
    identb = np.eye(128, dtype=np.float32).astype(ml_dtypes.bfloat16)

    def wtiles(W, sl):
        Wt = np.ascontiguousarray(W[sl, :].T)        # [D, e]
        return np.ascontiguousarray(
            Wt.reshape(8, 128, Wt.shape[1]).transpose(1, 0, 2))

    WoT = np.ascontiguousarray(np.asarray(Wo, dtype=np.float32).T)
    wo_t = np.ascontiguousarray(WoT.reshape(8, 128, D).transpose(1, 0, 2))

    in_maps = []
    for c in range(N_CORES):
        sl = slice(EC * c, EC * (c + 1))
        in_maps.append({
            "xt": xt_bf,
            "wq": wtiles(np.asarray(Wq, np.float32), sl).astype(
                ml_dtypes.bfloat16),
            "wk": wtiles(np.asarray(Wk, np.float32), sl).astype(
                ml_dtypes.bfloat16),
            "wv": wtiles(np.asarray(Wv, np.float32), sl).astype(
                ml_dtypes.bfloat16),
            "wo": wo_t.astype(ml_dtypes.bfloat16),
            "cosb": cosb,
            "sinb": sinb,
            "rotm": rotm,
            "trimask": tri,
            "identb": identb,
        })
    return in_maps


def kernel(x, Wq, Wk, Wv, Wo, token_positions):
    global last_results
    if "nc" not in _CACHE:
        _CACHE["nc"] = _build_program()
    nc = _CACHE["nc"]
    in_maps = _host_inputs(x, Wq, Wk, Wv, Wo, token_positions)
    res = bass_utils.run_bass_kernel_spmd(nc, in_maps, list(range(N_CORES)))
    last_results = res
    y = np.empty((NT, D), dtype=np.float32)
    for c in range(N_CORES):
        yc = res.results[c]["y"]
        for bb in range(B):
            for slot in range(2):
                g0 = 2048 * bb + 128 * (c + 8 * slot)
                y[g0:g0 + 128] = yc[256 * bb + 128 * slot:
                                    256 * bb + 128 * slot + 128]
    return y.reshape(B, T, D)


# revision 4
# speedup vs baseline: 1.5526x; 1.0467x over previous
"""Multi-head self-attention (RoPE, causal) on 8 Trainium2 NeuronCores.

Sharding: tensor-parallel over heads. Each core owns 2 of 16 heads:
  - QKV projections column-sharded (each core computes its 128 features)
  - attention per (batch, head) fully on-core; scores kept transposed
    [tk, tq] so softmax needs no PE transposes; exp on ACT (both heads in
    one instruction via a strided AP); denominator via a ones-row in V;
    1/denom via ACT ln/exp (batched per half-batch to avoid table
    reloads); broadcast via gpsimd partition_broadcast
  - two AllToAlls per batch (strided token blocks, halves) switch
    head-sharding to token-sharding; output projection per half-batch,
    overlapped with the next batch's attention
  - phase-1 chunks for batch b+1 and oproj for batch b-1 are interleaved
    into batch b's attention loop so the PE never starves

dtypes: bf16 for all matmul operands; fp32 PSUM and softmax statistics.
"""

import numpy as np
import ml_dtypes

import concourse.bacc as bacc
import concourse.mybir as mybir
import concourse.tile as tile
from concourse import bass_utils

F32 = mybir.dt.float32
BF16 = mybir.dt.bfloat16

B, T, D = 4, 2048, 1024
H, DH = 16, 64
N_CORES = 8
HPC = H // N_CORES            # heads per core = 2
EC = HPC * DH                 # feature slice per core = 128
NT = B * T                    # 8192 tokens
TPC = NT // N_CORES           # 1024 tokens per core
THETA = 10000.0
NBB = T // 128                # 16 tk blocks per batch

_CACHE = {}
last_results = None


def _build_program():
    nc = bacc.Bacc("TRN2", debug=False, target_bir_lowering=False,
                   num_devices=N_CORES)

    xt_d = nc.dram_tensor("xt", [128, 8, NT], BF16, kind="ExternalInput")
    wq_d = nc.dram_tensor("wq", [128, 8, EC], BF16, kind="ExternalInput")
    wk_d = nc.dram_tensor("wk", [128, 8, EC], BF16, kind="ExternalInput")
    wv_d = nc.dram_tensor("wv", [128, 8, EC], BF16, kind="ExternalInput")
    wo_d = nc.dram_tensor("wo", [128, 8, D], BF16, kind="ExternalInput")
    cos_d = nc.dram_tensor("cosb", [128, T], F32, kind="ExternalInput")
    sin_d = nc.dram_tensor("sinb", [128, T], F32, kind="ExternalInput")
    rotm_d = nc.dram_tensor("rotm", [128, 128], BF16, kind="ExternalInput")
    tri_d = nc.dram_tensor("trimask", [128, 128], BF16, kind="ExternalInput")
    id_d = nc.dram_tensor("identb", [128, 128], BF16, kind="ExternalInput")
    y_d = nc.dram_tensor("y", [TPC, D], F32, kind="ExternalOutput")

    with tile.TileContext(nc) as tc:
        with (
            tc.tile_pool(name="consts", bufs=1) as consts,
            tc.tile_pool(name="big", bufs=1) as big,
            tc.tile_pool(name="xp", bufs=2) as xp,
            tc.tile_pool(name="stage", bufs=2) as stage,
            tc.tile_pool(name="expp", bufs=4) as expp,
            tc.tile_pool(name="outp", bufs=2) as outp,
            tc.tile_pool(name="oall_p", bufs=2) as oall_p,
            tc.tile_pool(name="scp", bufs=2, space="PSUM") as scp,
            tc.tile_pool(name="pvp", bufs=1, space="PSUM") as pvp,
            tc.tile_pool(name="psC", bufs=2, space="PSUM") as psC,
            tc.tile_pool(name="dram", bufs=1, space="DRAM") as dram,
        ):
            # ---- constants ----
            cos_sb = consts.tile([128, T], F32)
            sin_sb = consts.tile([128, T], F32)
            rotm_sb = consts.tile([128, 128], BF16)
            tri_sb = consts.tile([128, 128], BF16)
            ident_sb = consts.tile([128, 128], BF16)
            nc.sync.dma_start(cos_sb[:], cos_d[:, :])
            nc.sync.dma_start(sin_sb[:], sin_d[:, :])
            nc.sync.dma_start(rotm_sb[:], rotm_d[:, :])
            nc.sync.dma_start(tri_sb[:], tri_d[:, :])
            nc.sync.dma_start(ident_sb[:], id_d[:, :])

            wq_sb = consts.tile([128, 8, EC], BF16)
            wk_sb = consts.tile([128, 8, EC], BF16)
            wv_sb = consts.tile([128, 8, EC], BF16)
            wo_sb = consts.tile([128, 8, D], BF16)
            nc.sync.dma_start(wq_sb[:], wq_d[:, :, :])
            nc.sync.dma_start(wk_sb[:], wk_d[:, :, :])
            nc.sync.dma_start(wv_sb[:], wv_d[:, :, :])
            nc.sync.dma_start(wo_sb[:], wo_d[:, :, :])

            # ---- persistent tensors ----
            qT = big.tile([128, NT], BF16, tag="qT")
            kT = big.tile([128, NT], BF16, tag="kT")
            vext = big.tile([128, HPC * B, NBB, 65], BF16, tag="vext")
            nc.vector.memset(vext[:, :, :, 64], 1.0)

            # two collectives per batch: half hf covers tk blocks 8*hf..8*hf+7
            a2a_in = [[dram.tile([N_CORES, 128, 128], BF16, tag=f"ai{b}{hf}",
                                 name=f"ai{b}{hf}") for hf in range(2)]
                      for b in range(B)]
            a2a_out = [[dram.tile([N_CORES, 128, 128], BF16, tag=f"ao{b}{hf}",
                                  name=f"ao{b}{hf}") for hf in range(2)]
                       for b in range(B)]

            xts = {}

            def xt_load(ci):
                xts[ci] = xp.tile([128, 8, 512], BF16, tag="x",
                                  name=f"xt{ci}")
                nc.sync.dma_start(xts[ci][:], xt_d[:, :, 512 * ci:
                                                   512 * ci + 512])

            # ---------- phase-1 chunk as a list of closures -------------
            def chunk_pieces(ci):
                t0 = 512 * ci
                bb = t0 // T
                s0 = t0 % T
                ps = []
                st = {}

                def proj_mm(w_sb, nm, ko):
                    def f():
                        if ko == 0:
                            st[nm] = psC.tile([128, 512], F32, tag="pp",
                                              name="p" + nm)
                        nc.tensor.matmul(st[nm], w_sb[:, ko, :],
                                         xts[ci][:, ko, :],
                                         start=(ko == 0), stop=(ko == 7))
                    return f

                def drain(nm):
                    def f():
                        st["raw" + nm] = stage.tile([128, 512], BF16,
                                                    tag="raw" + nm,
                                                    name="raw" + nm)
                        nc.scalar.copy(st["raw" + nm][:], st[nm])
                    return f

                def rot(nm):
                    def f():
                        st["rot" + nm] = psC.tile([128, 512], F32, tag="pp",
                                                  name="rot" + nm)
                        nc.tensor.matmul(st["rot" + nm], rotm_sb[:],
                                         st["raw" + nm][:],
                                         start=True, stop=True)
                    return f

                def comb(nm, dest):
                    def f1():
                        st["t2" + nm] = stage.tile([128, 512], F32,
                                                   tag="t2" + nm,
                                                   name="t2" + nm)
                        nc.vector.tensor_tensor(
                            st["t2" + nm][:], st["rot" + nm],
                            sin_sb[:, s0:s0 + 512], mybir.AluOpType.mult)

                    def f2():
                        st["t1" + nm] = stage.tile([128, 512], F32,
                                                   tag="t1" + nm,
                                                   name="t1" + nm)
                        nc.vector.tensor_tensor(
                            st["t1" + nm][:], st["raw" + nm][:],
                            cos_sb[:, s0:s0 + 512], mybir.AluOpType.mult)

                    def f3():
                        nc.vector.tensor_tensor(
                            dest[:, t0:t0 + 512], st["t1" + nm][:],
                            st["t2" + nm][:], mybir.AluOpType.add)
                    return [f1, f2, f3]

                def vtrans(h, bi):
                    def f():
                        pair = bb * HPC + h
                        jg = s0 // 128 + bi
                        tp = psC.tile([128, 512], F32, tag="pp",
                                      name="vtr").bitcast(BF16)[:, 0:64]
                        nc.tensor.transpose(
                            tp, st["rawv"][64 * h:64 * h + 64,
                                           128 * bi:128 * bi + 128],
                            ident_sb[64 * h:64 * h + 64,
                                     64 * h:64 * h + 64])
                        nc.vector.tensor_copy(vext[:, pair, jg, 0:64], tp)
                    return f

                for ko in range(8):
                    ps.append(proj_mm(wq_sb, "q", ko))
                ps.append(drain("q"))
                for ko in range(8):
                    ps.append(proj_mm(wk_sb, "k", ko))
                ps.append(rot("q"))
                ps.extend(comb("q", qT))
                ps.append(drain("k"))
                for ko in range(8):
                    ps.append(proj_mm(wv_sb, "v", ko))
                ps.append(rot("k"))
                ps.extend(comb("k", kT))
                ps.append(drain("v"))
                for h in range(HPC):
                    for bi in range(4):
                        ps.append(vtrans(h, bi))
                return ps

            # ---------- output projection for one half-batch -------------
            def oproj_pieces(bb, hf):
                # tokens: tk blocks {c + 8*hf} -> y rows 256*bb + 128*hf ..
                ps = []
                st = {}

                def load():
                    st["oall"] = oall_p.tile([128, 8, 128], BF16,
                                             tag=f"oall{hf}",
                                             name=f"oall{bb}{hf}")
                    nc.sync.dma_start(
                        st["oall"][:],
                        a2a_out[bb][hf][:].rearrange("s p t -> p s t"))

                def piece(eo):
                    def f():
                        ot = psC.tile([128, 512], F32, tag="pp", name="ot")
                        for ec in range(8):
                            nc.tensor.matmul(
                                ot, st["oall"][:, ec, :],
                                wo_sb[:, ec, 512 * eo:512 * eo + 512],
                                start=(ec == 0), stop=(ec == 7))
                        ys = outp.tile([128, 512], F32, tag="ys", name="ys")
                        nc.scalar.copy(ys[:], ot)
                        nc.sync.dma_start(
                            y_d[256 * bb + 128 * hf:256 * bb + 128 * hf + 128,
                                512 * eo:512 * eo + 512], ys[:])
                    return f

                ps.append(load)
                for eo in range(2):
                    ps.append(piece(eo))
                return ps

            # ---------- attention for one batch, with filler -------------
            def do_attn(bb, filler):
                fidx = [0]

                def pop_filler(k):
                    while k > 0 and fidx[0] < len(filler):
                        filler[fidx[0]]()
                        fidx[0] += 1
                        k -= 1

                tb0 = bb * T
                pair0 = bb * HPC

                for half in range(2):
                    unns = []
                    for q4 in (2 * half, 2 * half + 1):
                        jmax = 4 * q4 + 4
                        tq0 = tb0 + 512 * q4
                        pvt = [pvp.tile([65, 512], F32, tag=f"pv{hh}",
                                        name=f"pvt{hh}") for hh in range(2)]

                        def scores(j):
                            sc = scp.tile([128, 2, 512], F32, tag="sc",
                                          name="sc")
                            lo = max(0, 128 * j - 512 * q4)
                            for hh in range(2):
                                nc.tensor.matmul(
                                    sc[:, hh, lo:512],
                                    kT[64 * hh:64 * hh + 64,
                                       tb0 + 128 * j:tb0 + 128 * j + 128],
                                    qT[64 * hh:64 * hh + 64,
                                       tq0 + lo:tq0 + 512],
                                    start=True, stop=True)
                            return sc, lo

                        s_cur = scores(0)
                        prev = None
                        for j in range(jmax):
                            sc, lo = s_cur
                            ex = expp.tile([128, 2, 512], BF16, tag="ex",
                                           name="ex")
                            nc.scalar.activation(
                                ex[:, :, lo:512], sc[:, :, lo:512],
                                mybir.ActivationFunctionType.Exp,
                                scale=0.125)
                            if prev is not None:
                                pj, plo, pex = prev
                                for hh in range(2):
                                    nc.tensor.matmul(
                                        pvt[hh][:, plo:512],
                                        vext[:, pair0 + hh, pj, 0:65],
                                        pex[:, hh, plo:512],
                                        start=(pj == 0), stop=False)
                            if j + 1 < jmax:
                                s_cur = scores(j + 1)
                            if j >= 4 * q4:
                                d0 = 128 * (j - 4 * q4)
                                for hh in range(2):
                                    nc.vector.tensor_tensor(
                                        ex[:, hh, d0:d0 + 128],
                                        ex[:, hh, d0:d0 + 128],
                                        tri_sb[:], mybir.AluOpType.mult)
                            prev = (j, lo, ex)
                            pop_filler(5)
                        pj, plo, pex = prev
                        for hh in range(2):
                            nc.tensor.matmul(
                                pvt[hh][:, plo:512],
                                vext[:, pair0 + hh, pj, 0:65],
                                pex[:, hh, plo:512],
                                start=(pj == 0), stop=True)
                        # drain PSUM per head so the next quarter's PV can
                        # start as soon as possible
                        unn = outp.tile([65, 2, 512], F32, tag="unn",
                                        name="unn")
                        for hh in range(2):
                            nc.scalar.copy(unn[:, hh, :], pvt[hh][:])
                        unns.append((q4, unn))
                        pop_filler(4)

                    # ---- batched normalize + ship for this half ----
                    # ln then exp(-x) in place on the denominator rows,
                    # grouped so the ACT table only swaps twice per half.
                    for q4, unn in unns:
                        nc.scalar.activation(
                            unn[64:65, :, :], unn[64:65, :, :],
                            mybir.ActivationFunctionType.Ln)
                    for q4, unn in unns:
                        nc.scalar.activation(
                            unn[64:65, :, :], unn[64:65, :, :],
                            mybir.ActivationFunctionType.Exp, scale=-1.0)
                    for idx, (q4, unn) in enumerate(unns):
                        rec = outp.tile([1, 2, 512], F32, tag=f"rec{idx}",
                                        name="rec")
                        nc.sync.dma_start(rec[:], unn[64:65, :, :])
                        recb = outp.tile([64, 2, 512], F32, tag=f"recb{idx}",
                                         name="recb")
                        nc.gpsimd.partition_broadcast(recb[:], rec[:])
                        ao = outp.tile([64, 2, 512], BF16, tag=f"aot{idx}",
                                       name="aot")
                        nc.vector.scalar_tensor_tensor(
                            ao[:], unn[0:64, :, :], 1.0, recb[:],
                            mybir.AluOpType.mult, mybir.AluOpType.mult)
                        for hh in range(2):
                            for tb in range(4):
                                j16 = 4 * q4 + tb
                                dest = j16 % 8
                                hfi = j16 // 8
                                nc.sync.dma_start(
                                    a2a_in[bb][hfi][dest,
                                                    64 * hh:64 * hh + 64, :],
                                    ao[:, hh, 128 * tb:128 * tb + 128])
                        pop_filler(2)

                    nc.gpsimd.collective_compute(
                        "AllToAll", mybir.AluOpType.bypass,
                        replica_groups=[list(range(N_CORES))],
                        ins=[a2a_in[bb][half].opt()],
                        outs=[a2a_out[bb][half].opt()])
                    if bb == B - 1:
                        # last batch: its own oproj is the only filler left
                        filler.extend(oproj_pieces(bb, half))
                pop_filler(len(filler))

            # ================= main schedule ==========================
            xt_load(0)
            for ci in range(4):
                if ci + 1 < 16:
                    xt_load(ci + 1)
                for p in chunk_pieces(ci):
                    p()
            for bb in range(B):
                filler = []
                if bb + 1 < B:
                    for ci in range(4 * bb + 4, 4 * bb + 8):
                        filler.extend(chunk_pieces(ci))
                        if ci + 1 < 16:
                            filler.append(lambda c=ci + 1: xt_load(c))
                if bb - 1 >= 0:
                    for hf in range(2):
                        filler.extend(oproj_pieces(bb - 1, hf))
                do_attn(bb, filler)

    nc.compile()
    return nc


def _host_inputs(x, Wq, Wk, Wv, Wo, token_positions):
    """Per-core in_maps with transposed/tiled layouts."""
    x = np.asarray(x, dtype=np.float32)
    xt_bf = np.ascontiguousarray(
        x.reshape(NT, D).T.reshape(8, 128, NT).transpose(1, 0, 2)
    ).astype(ml_dtypes.bfloat16)

    pos = np.asarray(token_positions).astype(np.float64)
    inv_freq = 1.0 / (THETA ** (np.arange(0, DH, 2, dtype=np.float64) / DH))
    ang = pos[None, :] * inv_freq[:, None]          # [32, T]
    cos_p = np.cos(ang)
    sin_p = np.sin(ang)
    d_idx = (np.arange(128) % 64) // 2
    cosb = cos_p[d_idx, :].astype(np.float32)
    sinb = sin_p[d_idx, :].astype(np.float32)

    rotm = np.zeros((128, 128), dtype=np.float32)
    for i in range(64):
        rotm[2 * i + 1, 2 * i] = -1.0
        rotm[2 * i, 2 * i + 1] = 1.0
    rotm = rotm.astype(ml_dtypes.bfloat16)
    tri = np.tril(np.ones((128, 128), dtype=np.float32)).T  # [tk, tq]
    tri = tri.astype(ml_dtypes.bfloat16)
    identb = np.eye(128, dtype=np.float32).astype(ml_dtypes.bfloat16)

    def wtiles(W, sl):
        Wt = np.ascontiguousarray(W[sl, :].T)        # [D, e]
        return np.ascontiguousarray(
            Wt.reshape(8, 128, Wt.shape[1]).transpose(1, 0, 2))

    WoT = np.ascontiguousarray(np.asarray(Wo, dtype=np.float32).T)
    wo_t = np.ascontiguousarray(WoT.reshape(8, 128, D).transpose(1, 0, 2))

    in_maps = []
    for c in range(N_CORES):
        sl = slice(EC * c, EC * (c + 1))
        in_maps.append({
            "xt": xt_bf,
            "wq": wtiles(np.asarray(Wq, np.float32), sl).astype(
                ml_dtypes.bfloat16),
            "wk": wtiles(np.asarray(Wk, np.float32), sl).astype(
                ml_dtypes.bfloat16),
            "wv": wtiles(np.asarray(Wv, np.float32), sl).astype(
                ml_dtypes.bfloat16),
            "wo": wo_t.astype(ml_dtypes.bfloat16),
            "cosb": cosb,
            "sinb": sinb,
            "rotm": rotm,
            "trimask": tri,
            "identb": identb,
        })
    return in_maps


def kernel(x, Wq, Wk, Wv, Wo, token_positions):
    global last_results
    if "nc" not in _CACHE:
        _CACHE["nc"] = _build_program()
    nc = _CACHE["nc"]
    in_maps = _host_inputs(x, Wq, Wk, Wv, Wo, token_positions)
    res = bass_utils.run_bass_kernel_spmd(nc, in_maps, list(range(N_CORES)))
    last_results = res
    y = np.empty((NT, D), dtype=np.float32)
    for c in range(N_CORES):
        yc = res.results[c]["y"]
        for bb in range(B):
            for hf in range(2):
                g0 = 2048 * bb + 128 * (c + 8 * hf)
                y[g0:g0 + 128] = yc[256 * bb + 128 * hf:
                                    256 * bb + 128 * hf + 128]
    return y.reshape(B, T, D)
